# revision 5
# baseline (speedup 1.0000x reference)
"""Trainium2 Bass kernel: full (non-causal) multi-head attention.

Problem: B=2, S=2048, H=16, D=64, fp32 in/out.
  out[b,q,h,:] = softmax(Q K^T / sqrt(D))[q,:] @ V   per (b,h)

Strategy: attention is independent per (batch, head) pair. There are
B*H = 32 pairs; shard 4 pairs to each of the 8 NeuronCores
(head-parallel => zero inter-core communication). All sharding /
layout packing happens host-side in numpy (not timed); the NEFF per
core computes 4 full attention heads.

Per-core layout (host-prepared, bf16):
  qt  [128, 2*2048]  partition p<64 -> pair 2g d=p ; p>=64 -> pair 2g+1
  kt  [128, 2*2048]  same packing (transposed: partition = head dim)
  v1  [128, 4*16*65] V tiles [kb][128 k, 64 d] + a ones column (col 64)
                     -> PV matmul also accumulates the softmax row-sums.
  out [128, 4*16*64] fp32, partition = q % 128 within each q-block.

Per (pair, k-block kb of 128 keys):
  S^T[kb] = (K^T tile).T @ Q^T       (PE, bf16, contract=64, row-tiled)
  P^T[kb] = exp(S^T[kb] * 1/8)       (ScalarE, PSUM->SBUF bf16)
  O[qb]  += P^T[kb][:,qb].T @ V1[kb] (PE, bf16, contract=128, accum PSUM)
Then O[:, :64] * 1/O[:, 64] (DVE reciprocal + per-partition scalar mul).
"""

import sys

if '/opt/trn_rl_repo' not in sys.path:
    sys.path.insert(0, '/opt/trn_rl_repo')

import numpy as np
import ml_dtypes

from concourse import bacc, tile, mybir
from concourse.bass_utils import run_bass_kernel_spmd

B, S, H, D = 2, 2048, 16, 64
N_CORES = 8
PAIRS = B * H              # 32 (b,h) pairs
PPC = PAIRS // N_CORES     # 4 pairs per core
NKB = S // 128             # 16 k-blocks
NQB = S // 128             # 16 q-blocks
SCALE = 1.0 / np.sqrt(D)   # 0.125

BF16 = mybir.dt.bfloat16
F32 = mybir.dt.float32


def _build_kernel():
    nc = bacc.Bacc("TRN2", target_bir_lowering=False, debug=False,
                   num_devices=N_CORES)
    qt_ap = nc.dram_tensor("qt", [128, 2 * S], BF16, kind="ExternalInput").ap()
    kt_ap = nc.dram_tensor("kt", [128, 2 * S], BF16, kind="ExternalInput").ap()
    v1_ap = nc.dram_tensor("v1", [128, PPC * NKB * 65], BF16,
                           kind="ExternalInput").ap()
    out_ap = nc.dram_tensor("out", [128, PPC * NQB * 64], F32,
                            kind="ExternalOutput").ap()

    with tile.TileContext(nc) as tc:
        import contextlib
        with contextlib.ExitStack() as ctx:
            in_pool = ctx.enter_context(tc.tile_pool(name="inp", bufs=1))
            pt_pool = ctx.enter_context(tc.tile_pool(name="pt", bufs=20))
            osb_pool = ctx.enter_context(tc.tile_pool(name="osb", bufs=2))
            rec_pool = ctx.enter_context(tc.tile_pool(name="rec", bufs=2))
            st_pool = ctx.enter_context(
                tc.tile_pool(name="st", bufs=2, space="PSUM"))
            o_pool = ctx.enter_context(
                tc.tile_pool(name="o", bufs=4, space="PSUM"))

            qt_sb = in_pool.tile([128, 2 * S], BF16)
            kt_sb = in_pool.tile([128, 2 * S], BF16)
            v1_sb = in_pool.tile([128, PPC * NKB * 65], BF16)
            nc.sync.dma_start(out=qt_sb[:], in_=qt_ap[:])
            nc.sync.dma_start(out=kt_sb[:], in_=kt_ap[:])
            nc.sync.dma_start(out=v1_sb[:], in_=v1_ap[:])

            for p in range(PPC):
                g, h = p // 2, p % 2
                hs = slice(64 * h, 64 * h + 64)
                gq = 2048 * g

                # --- S^T + exp for all 16 k-blocks (buffered in SBUF) ---
                pts = []
                for kb in range(NKB):
                    ktile = kt_sb[hs, gq + 128 * kb: gq + 128 * kb + 128]
                    pt = pt_pool.tile([128, S], BF16, name=f"pt_{p}_{kb}",
                                      tag="pt")
                    for half in range(2):
                        st = st_pool.tile([128, 1024], F32,
                                          name=f"st_{p}_{kb}_{half}", tag="st")
                        for j in range(2):
                            q0 = gq + 1024 * half + 512 * j
                            nc.tensor.matmul(
                                st[:, 512 * j: 512 * j + 512],
                                lhsT=ktile,
                                rhs=qt_sb[hs, q0: q0 + 512],
                                start=True, stop=True)
                        nc.scalar.activation(
                            pt[:, 1024 * half: 1024 * half + 1024],
                            st[:],
                            mybir.ActivationFunctionType.Exp,
                            scale=float(SCALE))
                    pts.append(pt)

                # --- PV in 4 batches of 4 q-blocks (one PSUM bank each) ---
                osb = osb_pool.tile([128, NQB * 64], F32)
                for bt in range(4):
                    o4 = [o_pool.tile([128, 65], F32, tag="o",
                                      name=f"o_{p}_{bt}_{i}")
                          for i in range(4)]
                    for kb in range(NKB):
                        vt = v1_sb[:, 1040 * p + 65 * kb:
                                   1040 * p + 65 * kb + 65]
                        for jj in range(4):
                            qb = 4 * bt + jj
                            nc.tensor.matmul(
                                o4[jj][:],
                                lhsT=pts[kb][:, 128 * qb: 128 * qb + 128],
                                rhs=vt,
                                start=(kb == 0), stop=(kb == NKB - 1),
                                skip_group_check=True)
                    # normalize: out[:, q, d] = o[:, q, d] / o[:, q, 64]
                    rec = rec_pool.tile([128, 4], F32, name=f"rec_{p}_{bt}",
                                        tag="rec")
                    for jj in range(4):
                        nc.vector.reciprocal(rec[:, jj: jj + 1],
                                             o4[jj][:, 64: 65])
                    for jj in range(4):
                        qb = 4 * bt + jj
                        nc.vector.tensor_scalar_mul(
                            osb[:, 64 * qb: 64 * qb + 64],
                            o4[jj][:, 0: 64],
                            rec[:, jj: jj + 1])
                nc.sync.dma_start(
                    out=out_ap[:, 1024 * p: 1024 * p + 1024], in_=osb[:])

    nc.compile()
    return nc


_NC_CACHE = {}


def _get_nc():
    if "nc" not in _NC_CACHE:
        _NC_CACHE["nc"] = _build_kernel()
    return _NC_CACHE["nc"]


def _shard_inputs(query, key, value):
    """Full [B,S,H,D] f32 -> per-core bf16 packed arrays."""
    bf = ml_dtypes.bfloat16
    # [B,S,H,D] -> [B,H,S,D] -> [32, S, D]
    q = np.ascontiguousarray(query.transpose(0, 2, 1, 3)).reshape(PAIRS, S, D)
    k = np.ascontiguousarray(key.transpose(0, 2, 1, 3)).reshape(PAIRS, S, D)
    v = np.ascontiguousarray(value.transpose(0, 2, 1, 3)).reshape(PAIRS, S, D)
    in_maps = []
    for c in range(N_CORES):
        sl = slice(PPC * c, PPC * (c + 1))
        qc, kc, vc = q[sl], k[sl], v[sl]
        # transposed: [4, S, D] -> [4, D, S] -> [2, 128, S] -> [128, 2*S]
        qt = qc.transpose(0, 2, 1).reshape(2, 128, S).transpose(1, 0, 2) \
            .reshape(128, 2 * S)
        kt = kc.transpose(0, 2, 1).reshape(2, 128, S).transpose(1, 0, 2) \
            .reshape(128, 2 * S)
        # v: [4, S, D] -> [4, 16, 128, D] -> ones col -> [128, 4*16*65]
        v4 = vc.reshape(PPC, NKB, 128, D)
        v1 = np.ones((PPC, NKB, 128, D + 1), np.float32)
        v1[:, :, :, :D] = v4
        v1 = v1.transpose(2, 0, 1, 3).reshape(128, PPC * NKB * 65)
        in_maps.append({
            "qt": np.ascontiguousarray(qt).astype(bf),
            "kt": np.ascontiguousarray(kt).astype(bf),
            "v1": np.ascontiguousarray(v1).astype(bf),
        })
    return in_maps


def _unshard_output(results):
    """Per-core out [128, 4*16*64] f32 -> full [B,S,H,D] f32."""
    outs = []
    for c in range(N_CORES):
        o = results[c]["out"].reshape(128, PPC, NQB, D)
        outs.append(o.transpose(1, 2, 0, 3).reshape(PPC, S, D))
    full = np.concatenate(outs, axis=0)          # [32, S, D]
    full = full.reshape(B, H, S, D).transpose(0, 2, 1, 3)  # [B,S,H,D]
    return np.ascontiguousarray(full)


def kernel(query, key, value):
    nc = _get_nc()
    in_maps = _shard_inputs(np.asarray(query, np.float32),
                            np.asarray(key, np.float32),
                            np.asarray(value, np.float32))
    res = run_bass_kernel_spmd(nc, in_maps, core_ids=list(range(N_CORES)))
    return _unshard_output(res.results)


if __name__ == "__main__":
    rng = np.random.default_rng(0)
    q = rng.standard_normal((B, S, H, D), np.float32)
    k = rng.standard_normal((B, S, H, D), np.float32)
    v = rng.standard_normal((B, S, H, D), np.float32)
    o = kernel(query=q, key=k, value=v)
    print("out", o.shape, o.dtype, np.abs(o).mean())


# revision 8
# speedup vs baseline: 19.8590x; 19.8590x over previous
"""Trainium2 Bass kernel: full (non-causal) multi-head attention.

Problem: B=2, S=2048, H=16, D=64, fp32 in/out.
  out[b,q,h,:] = softmax(Q K^T / sqrt(D))[q,:] @ V   per (b,h)

Strategy: attention is independent per (batch, head) pair. There are
B*H = 32 pairs; shard 4 pairs to each of the 8 NeuronCores
(head-parallel => zero inter-core communication). All sharding /
layout packing happens host-side in numpy (not timed); the NEFF per
core computes 4 full attention heads.

Per-core layout (host-prepared, bf16):
  qt  [128, 2*2048]  partition p<64 -> pair 2g d=p ; p>=64 -> pair 2g+1
  kt  [128, 2*2048]  same packing (transposed: partition = head dim)
  v1  [128, 4*16*65] V tiles [kb][128 k, 64 d] + a ones column (col 64)
                     -> PV matmul also accumulates the softmax row-sums.
  out [128, 4*16*64] fp32, partition = q % 128 within each q-block.

Per (pair, k-block kb of 128 keys):
  S^T[kb] = (K^T tile).T @ Q^T       (PE, bf16, contract=64, row-tiled)
  P^T[kb] = exp(S^T[kb] * 1/8)       (ScalarE, PSUM->SBUF bf16)
  O[qb]  += P^T[kb][:,qb].T @ V1[kb] (PE, bf16, contract=128, accum PSUM)
Then O[:, :64] * 1/O[:, 64] (DVE reciprocal + per-partition scalar mul).
"""

import sys

if '/opt/trn_rl_repo' not in sys.path:
    sys.path.insert(0, '/opt/trn_rl_repo')

import numpy as np
import ml_dtypes

from concourse import bacc, tile, mybir
from concourse.bass_utils import run_bass_kernel_spmd

B, S, H, D = 2, 2048, 16, 64
N_CORES = 8
PAIRS = B * H              # 32 (b,h) pairs
PPC = PAIRS // N_CORES     # 4 pairs per core
NKB = S // 128             # 16 k-blocks
NQB = S // 128             # 16 q-blocks
SCALE = 1.0 / np.sqrt(D)   # 0.125

BF16 = mybir.dt.bfloat16
F32 = mybir.dt.float32


def _build_kernel(reps=1):
    """reps>1 repeats the whole computation in one NEFF (timing use only)."""
    nc = bacc.Bacc("TRN2", target_bir_lowering=False, debug=False,
                   num_devices=N_CORES)
    qt_ap = nc.dram_tensor("qt", [128, 2 * S], BF16, kind="ExternalInput").ap()
    kt_ap = nc.dram_tensor("kt", [128, 2 * S], BF16, kind="ExternalInput").ap()
    v1_ap = nc.dram_tensor("v1", [128, PPC * NKB * 65], BF16,
                           kind="ExternalInput").ap()
    out_ap = nc.dram_tensor("out", [128, PPC * NQB * 64], F32,
                            kind="ExternalOutput").ap()

    with tile.TileContext(nc) as tc:
        import contextlib
        with contextlib.ExitStack() as ctx:
            in_pool = ctx.enter_context(tc.tile_pool(name="inp", bufs=1))
            pt_pool = ctx.enter_context(tc.tile_pool(name="pt", bufs=20))
            osb_pool = ctx.enter_context(tc.tile_pool(name="osb", bufs=2))
            rec_pool = ctx.enter_context(tc.tile_pool(name="rec", bufs=2))
            st_pool = ctx.enter_context(
                tc.tile_pool(name="st", bufs=2, space="PSUM"))
            o_pool = ctx.enter_context(
                tc.tile_pool(name="o", bufs=4, space="PSUM"))

            qt_sb = in_pool.tile([128, 2 * S], BF16)
            kt_sb = in_pool.tile([128, 2 * S], BF16)
            v1_sb = in_pool.tile([128, PPC * NKB * 65], BF16)
            nc.sync.dma_start(out=qt_sb[:], in_=qt_ap[:])
            nc.sync.dma_start(out=kt_sb[:], in_=kt_ap[:])
            nc.sync.dma_start(out=v1_sb[:], in_=v1_ap[:])

            for rep in range(reps):
              for p in range(PPC):
                g, h = p // 2, p % 2
                hs = slice(64 * h, 64 * h + 64)
                gq = 2048 * g

                # --- S^T + exp for all 16 k-blocks (buffered in SBUF) ---
                pts = []
                for kb in range(NKB):
                    ktile = kt_sb[hs, gq + 128 * kb: gq + 128 * kb + 128]
                    pt = pt_pool.tile([128, S], BF16, name=f"pt_{p}_{kb}",
                                      tag="pt")
                    for half in range(2):
                        st = st_pool.tile([128, 1024], F32,
                                          name=f"st_{p}_{kb}_{half}", tag="st")
                        for j in range(2):
                            q0 = gq + 1024 * half + 512 * j
                            nc.tensor.matmul(
                                st[:, 512 * j: 512 * j + 512],
                                lhsT=ktile,
                                rhs=qt_sb[hs, q0: q0 + 512],
                                start=True, stop=True)
                        nc.scalar.activation(
                            pt[:, 1024 * half: 1024 * half + 1024],
                            st[:],
                            mybir.ActivationFunctionType.Exp,
                            scale=float(SCALE))
                    pts.append(pt)

                # --- PV in 4 batches of 4 q-blocks (one PSUM bank each) ---
                osb = osb_pool.tile([128, NQB * 64], F32)
                for bt in range(4):
                    o4 = [o_pool.tile([128, 65], F32, tag="o",
                                      name=f"o_{p}_{bt}_{i}")
                          for i in range(4)]
                    for kb in range(NKB):
                        vt = v1_sb[:, 1040 * p + 65 * kb:
                                   1040 * p + 65 * kb + 65]
                        for jj in range(4):
                            qb = 4 * bt + jj
                            nc.tensor.matmul(
                                o4[jj][:],
                                lhsT=pts[kb][:, 128 * qb: 128 * qb + 128],
                                rhs=vt,
                                start=(kb == 0), stop=(kb == NKB - 1),
                                skip_group_check=True)
                    # normalize: out[:, q, d] = o[:, q, d] / o[:, q, 64]
                    rec = rec_pool.tile([128, 4], F32, name=f"rec_{p}_{bt}",
                                        tag="rec")
                    for jj in range(4):
                        nc.vector.reciprocal(rec[:, jj: jj + 1],
                                             o4[jj][:, 64: 65])
                    for jj in range(4):
                        qb = 4 * bt + jj
                        nc.vector.tensor_scalar_mul(
                            osb[:, 64 * qb: 64 * qb + 64],
                            o4[jj][:, 0: 64],
                            rec[:, jj: jj + 1])
                nc.sync.dma_start(
                    out=out_ap[:, 1024 * p: 1024 * p + 1024], in_=osb[:])

    nc.compile()
    return nc


_NC_CACHE = {}


def _get_nc(reps=1):
    key = ("nc", reps)
    if key not in _NC_CACHE:
        _NC_CACHE[key] = _build_kernel(reps)
    return _NC_CACHE[key]


def _shard_inputs(query, key, value):
    """Full [B,S,H,D] f32 -> per-core bf16 packed arrays."""
    bf = ml_dtypes.bfloat16
    # [B,S,H,D] -> [B,H,S,D] -> [32, S, D]
    q = np.ascontiguousarray(query.transpose(0, 2, 1, 3)).reshape(PAIRS, S, D)
    k = np.ascontiguousarray(key.transpose(0, 2, 1, 3)).reshape(PAIRS, S, D)
    v = np.ascontiguousarray(value.transpose(0, 2, 1, 3)).reshape(PAIRS, S, D)
    in_maps = []
    for c in range(N_CORES):
        sl = slice(PPC * c, PPC * (c + 1))
        qc, kc, vc = q[sl], k[sl], v[sl]
        # transposed: [4, S, D] -> [4, D, S] -> [2, 128, S] -> [128, 2*S]
        qt = qc.transpose(0, 2, 1).reshape(2, 128, S).transpose(1, 0, 2) \
            .reshape(128, 2 * S)
        kt = kc.transpose(0, 2, 1).reshape(2, 128, S).transpose(1, 0, 2) \
            .reshape(128, 2 * S)
        # v: [4, S, D] -> [4, 16, 128, D] -> ones col -> [128, 4*16*65]
        v4 = vc.reshape(PPC, NKB, 128, D)
        v1 = np.ones((PPC, NKB, 128, D + 1), np.float32)
        v1[:, :, :, :D] = v4
        v1 = v1.transpose(2, 0, 1, 3).reshape(128, PPC * NKB * 65)
        in_maps.append({
            "qt": np.ascontiguousarray(qt).astype(bf),
            "kt": np.ascontiguousarray(kt).astype(bf),
            "v1": np.ascontiguousarray(v1).astype(bf),
        })
    return in_maps


def _unshard_output(results):
    """Per-core out [128, 4*16*64] f32 -> full [B,S,H,D] f32."""
    outs = []
    for c in range(N_CORES):
        o = results[c]["out"].reshape(128, PPC, NQB, D)
        outs.append(o.transpose(1, 2, 0, 3).reshape(PPC, S, D))
    full = np.concatenate(outs, axis=0)          # [32, S, D]
    full = full.reshape(B, H, S, D).transpose(0, 2, 1, 3)  # [B,S,H,D]
    return np.ascontiguousarray(full)


def kernel(query, key, value):
    nc = _get_nc()
    in_maps = _shard_inputs(np.asarray(query, np.float32),
                            np.asarray(key, np.float32),
                            np.asarray(value, np.float32))
    res = run_bass_kernel_spmd(nc, in_maps, core_ids=list(range(N_CORES)))
    return _unshard_output(res.results)


if __name__ == "__main__":
    rng = np.random.default_rng(0)
    q = rng.standard_normal((B, S, H, D), np.float32)
    k = rng.standard_normal((B, S, H, D), np.float32)
    v = rng.standard_normal((B, S, H, D), np.float32)
    o = kernel(query=q, key=k, value=v)
    print("out", o.shape, o.dtype, np.abs(o).mean())
